# revision 44
# baseline (speedup 1.0000x reference)
"""Correlation-volume kernel for Trainium2 (8 NeuronCores, SPMD).

Problem: inputs (B=4, N=2, C=128, H=128, W=128) fp32.
  q = floor(inputs * 1e10) / 1e10  (straight-through quantization, fp32)
  src = q[:, 0], tgt = q[:, 1]
  out[b, dy*21+dx, h, w] = mean_c src[b,c,h,w] * tgt[b,c,h+dy-10,w+dx-10]
  (zero padding outside), out shape (4, 441, 128, 128) fp32.

Strategy (final — fp16, ragged banded dump, port-scattered r-groups):
  - Shard batch(4) x H-half(2) across 8 cores, data parallel, no
    collectives. ~72us HW exec vs 131us for the fp32 dense-dump baseline.
  - Host precomputes q, casts fp16, blocks src into 128-pixel stationary
    tiles of 8 pixel-rows x 16 pixel-cols, zero-pads tgt rows only
    (84 x 128); one packed fp16 input per core, loaded in 9 chunks
    interleaved with src so compute starts after ~1.5MB.
  - Pixel -> partition map pi(r,c) = (r%4) + 64*(r//4) + 4c, so each
    pixel-row group's 16 partitions {p0(r)+4c} touch 8 SBUF AXI ports
    during the dump (a dense 16-partition group only reaches 4).
  - Device: per block, 2 fp16 matmuls (K=C=128, M=128 px, N=14 tgt rows
    x 26-36 ragged window cols) -> one 2-bank PSUM tile fp32; a single
    merged 4D DVE/ACT cast-copy per block -> fp16 staging, window-row-
    major [t:28][g:rows][wb][v] so each r-group's band (window rows
    r..r+20) is one contiguous 11-22KB run per partition.
  - Banded dump in supersteps of [1,1,2,2,1,1] hb rows (early start,
    small tail): one DMA per r-group spread over the sync HWDGE +
    gpsimd SWDGE (+ scalar on the tail) rings. 11.5MB/core total vs
    33MB dense fp32.
  - Host extracts (dy, dx) with per-wb strided views (the shear is
    unexpressible on-chip: BIR forbids mixed partition steps) and
    zeroes the structurally-out-of-image entries.
"""

import sys

if "/opt/trn_rl_repo" not in sys.path:
    sys.path.insert(0, "/opt/trn_rl_repo")

import numpy as np

B, NIN, C, H, W = 4, 2, 128, 128, 128
KH = KW = 21
QS = np.float32(1e10)
HHALF = 64            # rows per core
BH, BW = 8, 16        # pixel block (M = 128), partition = r*16 + c
NBH, NBW = HHALF // BH, W // BW      # 8, 8
RN = 14               # target rows per matmul (2 matmuls -> 28 = BH + 20)
WN = BW + 20          # 36 target cols per interior block
TR = BH + 20          # 28 window rows per block
# ragged window cols: edge blocks only cover the in-image part
WNS = [
    min(W, wb * BW + 26) - max(0, wb * BW - 10) for wb in range(NBW)
]                     # [26, 36*6, 26]
WC0 = [max(0, wb * BW - 10) for wb in range(NBW)]
WOFF = np.cumsum([0] + WNS).tolist()  # col offset inside a window row
ROWV = WOFF[-1]       # 268 elems per window row across a block row
SROW_F = TR * ROWV    # 7504 elems per partition per hb row
TROWS, TCOLS = HHALF + 20, W        # 84 rows (padded), 128 cols (no pad)
SRC_F = HHALF * W                    # 8192
TGT_F = TROWS * TCOLS                # 10752
PACK_F = SRC_F + TGT_F
NSS = NBH // 2                       # (legacy) 4 supersteps of 2 hb rows
SS_ROWS = [1, 1, 1, 1, 1, 1, 1, 1]   # hb rows per superstep
SS_HB0 = [0, 1, 2, 3, 4, 5, 6, 7]    # first hb of each superstep
# per-superstep band size per (r, c): 21 * g * ROWV elems
SS_BAND = [KH * g * ROWV for g in SS_ROWS]
SS_OFF = []
_o = 0
for _ss, _g in enumerate(SS_ROWS):
    SS_OFF.append(_o)
    _o += BH * BW * SS_BAND[_ss]
OUT_TOTAL = _o                       # 6193152 elems

# pixel (r, c) -> partition pi = (r%4) + 64*(r//4) + 4*c, so each r-group's
# 16 partitions {p0(r)+4c} hit 8 distinct SBUF AXI ports during the dump.
# _PIX_INV[m] = r*BW + c of the pixel stored in partition m.
_PIX_INV = np.array(
    [((m % 4) + 4 * (m // 64)) * BW + (m % 64) // 4 for m in range(128)],
    dtype=np.int64,
)

_nc_cache = None


def _build_nc():
    from contextlib import ExitStack

    from concourse import bacc, mybir, tile
    from concourse._compat import with_exitstack
    from concourse.ap import AP

    nc = bacc.Bacc("TRN2")
    pack = nc.declare_dram_parameter(
        "pack", [C, PACK_F], mybir.dt.float16, isOutput=False
    )
    # flat output: per superstep ss with g rows, 8 r-groups x 16 c x
    # (21 * g * 288) band elems
    out = nc.declare_dram_parameter(
        "out", [OUT_TOTAL], mybir.dt.float16, isOutput=True
    )

    # input chunks interleaved so each superstep's data lands just in time;
    # the first chunk is exactly what superstep 0 row 0 needs, to cut lead-in
    tgt_c = [
        (SRC_F + t0 * TCOLS, SRC_F + t1 * TCOLS)
        for t0, t1 in ((0, 28), (28, 44), (44, 60), (60, 76), (76, TROWS))
    ]
    assert TCOLS == W
    src_c = [(ss * 2048, (ss + 1) * 2048) for ss in range(NSS)]
    chunk_order = [
        tgt_c[0], src_c[0], tgt_c[1], src_c[1],
        tgt_c[2], src_c[2], tgt_c[3], src_c[3], tgt_c[4],
    ]

    @with_exitstack
    def kern(ctx: ExitStack, tc: tile.TileContext):
        nc = tc.nc
        sbp = ctx.enter_context(tc.tile_pool(name="inp", bufs=1))
        psp = ctx.enter_context(tc.tile_pool(name="ps", bufs=4, space="PSUM"))
        stg1 = ctx.enter_context(tc.tile_pool(name="stg1", bufs=2))
        stg2 = ctx.enter_context(tc.tile_pool(name="stg2", bufs=2))

        pk = sbp.tile([C, PACK_F], mybir.dt.float16, tag="pk")
        for lo, hi in chunk_order:
            nc.sync.dma_start(pk[:, lo:hi], pack[:, lo:hi])

        src2 = pk[:, 0:SRC_F]
        tgt3 = pk[:, SRC_F:].rearrange("c (t v) -> c t v", t=TROWS)

        copy_engines = [
            lambda d, s: nc.vector.tensor_copy(d, s),
            lambda d, s: nc.scalar.copy(d, s),
        ]
        ecnt = 0
        for ss, ng in enumerate(SS_ROWS):
            # staging layout per partition: [t:28][g:ng][wb:8][v:36]
            rowv2 = ng * ROWV
            srow = (stg1 if ng == 1 else stg2).tile(
                [128, ng * SROW_F], mybir.dt.float16
            )
            base = srow[:]
            pstride = base.ap[0][0]
            assert pstride == ng * SROW_F, (pstride, ng * SROW_F)
            for g in range(ng):
                hb = SS_HB0[ss] + g
                t0 = hb * BH
                for wb in range(NBW):
                    blk = hb * NBW + wb
                    wns, c0 = WNS[wb], WC0[wb]
                    nmm = RN * wns
                    lhs = src2[:, blk * 128 : (blk + 1) * 128]
                    ps = psp.tile([128, 1024], mybir.dt.float32)
                    nc.tensor.matmul(
                        ps[:, 0:nmm], lhs,
                        tgt3[:, t0 : t0 + RN, c0 : c0 + wns],
                        start=True, stop=True,
                    )
                    nc.tensor.matmul(
                        ps[:, 512 : 512 + nmm], lhs,
                        tgt3[:, t0 + RN : t0 + 2 * RN, c0 : c0 + wns],
                        start=True, stop=True,
                    )
                    # merged cast-copy: (p, half:2, t:14, v:wns)
                    src_ap = AP(
                        ps[:].tensor, ps[:].offset,
                        [[ps[:].ap[0][0], 128], [512, 2], [wns, RN], [1, wns]],
                    )
                    dst_ap = AP(
                        base.tensor,
                        base.offset + g * ROWV + WOFF[wb],
                        [[pstride, 128], [RN * rowv2, 2], [rowv2, RN], [1, wns]],
                    )
                    copy_engines[ecnt % 2](dst_ap, src_ap)
                    ecnt += 1
            # banded dump: r-group = 16 partitions {p0(r) + 4c}, ng rows
            # interleaved -> one contiguous (21*ng*288) run per partition.
            # The stride-4 partition scatter makes each DMA touch 8 SBUF
            # AXI ports (a dense 16-partition group only reaches 4).
            last = ss == len(SS_ROWS) - 1
            for r in range(BH):
                p0 = (r % 4) + 64 * (r // 4)
                band = AP(
                    base.tensor,
                    base.offset + p0 * pstride + r * rowv2,
                    [[4 * pstride, BW], [1, KH * rowv2]],
                )
                o0 = SS_OFF[ss] + r * BW * SS_BAND[ss]
                dst = out[o0 : o0 + BW * SS_BAND[ss]].rearrange(
                    "(c v) -> c v", c=BW
                )
                # spread across HWDGE(sync) + SWDGE(gpsimd) rings; on the
                # last superstep the scalar HWDGE ring joins (copies done)
                if last:
                    eng = (nc.sync, nc.gpsimd, nc.scalar)[r % 3]
                else:
                    eng = nc.sync if r % 2 == 0 else nc.gpsimd
                eng.dma_start(dst, band)

    with tile.TileContext(nc) as tc:
        kern(tc)
    nc.finalize()
    return nc


def _get_nc():
    global _nc_cache
    if _nc_cache is None:
        _nc_cache = _build_nc()
    return _nc_cache


def _pack_inputs(q: np.ndarray) -> list[dict]:
    """Per-core packed fp16 input: blocked src + zero-padded tgt."""
    in_maps = []
    for core in range(8):
        b, half = core // 2, core % 2
        h0 = half * HHALF
        src = q[b, 0, :, h0 : h0 + HHALF, :]            # (C, 64, 128)
        srcb = (
            src.reshape(C, NBH, BH, NBW, BW)
            .transpose(0, 1, 3, 2, 4)                   # (C, hb, wb, r, c)
            .reshape(C, NBH * NBW, BH * BW)[:, :, _PIX_INV]
            .reshape(C, SRC_F)
        )
        tgt = np.zeros((C, TROWS, TCOLS), np.float16)
        lo, hi = h0 - 10, h0 + HHALF + 10
        clo, chi = max(lo, 0), min(hi, H)
        tgt[:, clo - lo : chi - lo, :] = q[b, 1, :, clo:chi, :]
        pack = np.concatenate([srcb, tgt.reshape(C, TGT_F)], axis=1)
        in_maps.append({"pack": np.ascontiguousarray(pack)})
    return in_maps


def _unscramble(results: list[dict]) -> np.ndarray:
    """Extract the valid (dy, dx) band from each core's banded dump."""
    out = np.empty((B, KH * KW, H, W), np.float32)
    for core in range(8):
        b, half = core // 2, core % 2
        h0 = half * HHALF
        flat = np.asarray(results[core]["out"]).astype(np.float32)
        oc = np.zeros((KH, KW, HHALF, W), np.float32)
        for ss, ng in enumerate(SS_ROWS):
            # arr[r, c, u], u = dy*(ng*ROWV) + g*ROWV + WOFF[wb] + (c+dx-lo)
            arr = np.ascontiguousarray(
                flat[SS_OFF[ss] : SS_OFF[ss] + BH * BW * SS_BAND[ss]]
            ).reshape(BH, BW, SS_BAND[ss])
            # pad the band axis so edge-block shears stay in bounds
            arr = np.pad(arr, ((0, 0), (0, 0), (10, 10)))
            s_r, s_c, s_v = arr.strides
            h_lo = SS_HB0[ss] * BH
            for wb in range(NBW):
                lo_adj = wb * BW - 10 - WC0[wb]  # -10 for wb=0 else 0
                V = np.lib.stride_tricks.as_strided(
                    arr[:, :, 10 + WOFF[wb] + lo_adj :],
                    shape=(ng, BH, BW, KH, KW),
                    strides=(
                        ROWV * s_v, s_r, s_c + s_v,
                        ng * ROWV * s_v, s_v,
                    ),
                )
                # [g, r, c, dy, dx] -> [dy, dx, g, r, c]
                blkv = V.transpose(3, 4, 0, 1, 2).reshape(
                    KH, KW, ng * BH, BW
                )
                oc[:, :, h_lo : h_lo + ng * BH, wb * BW : (wb + 1) * BW] = blkv
        # zero structurally-invalid cols (displaced outside the image)
        wcol = np.arange(W)[None, :]
        dxs = np.arange(KW)[:, None]
        invalid = (wcol + dxs - 10 < 0) | (wcol + dxs - 10 >= W)  # [dx, w]
        oc *= ~invalid[None, :, None, :]
        out[b, :, h0 : h0 + HHALF, :] = oc.reshape(KH * KW, HHALF, W)
    out *= np.float32(1.0 / C)
    return out


def _run(inputs: np.ndarray, trace: bool = False, trace_kwargs: dict | None = None):
    from concourse.bass_utils import run_bass_kernel_spmd

    x = np.asarray(inputs, dtype=np.float32)
    assert x.shape == (B, NIN, C, H, W), x.shape
    q = (np.floor(x * QS) / QS).astype(np.float16)
    in_maps = _pack_inputs(q)
    nc = _get_nc()
    res = run_bass_kernel_spmd(
        nc, in_maps, core_ids=list(range(8)), trace=trace,
        **(trace_kwargs or {}),
    )
    out = _unscramble(res.results)
    return out, res


def kernel(inputs: np.ndarray) -> np.ndarray:
    out, _ = _run(inputs, trace=False)
    return out


# revision 45
# speedup vs baseline: 1.0400x; 1.0400x over previous
"""Correlation-volume kernel for Trainium2 (8 NeuronCores, SPMD).

Problem: inputs (B=4, N=2, C=128, H=128, W=128) fp32.
  q = floor(inputs * 1e10) / 1e10  (straight-through quantization, fp32)
  src = q[:, 0], tgt = q[:, 1]
  out[b, dy*21+dx, h, w] = mean_c src[b,c,h,w] * tgt[b,c,h+dy-10,w+dx-10]
  (zero padding outside), out shape (4, 441, 128, 128) fp32.

Strategy (final — fp16, ragged banded dump, port-scattered r-groups):
  - Shard batch(4) x H-half(2) across 8 cores, data parallel, no
    collectives. ~72us HW exec vs 131us for the fp32 dense-dump baseline.
  - Host precomputes q, casts fp16, blocks src into 128-pixel stationary
    tiles of 8 pixel-rows x 16 pixel-cols, zero-pads tgt rows only
    (84 x 128); one packed fp16 input per core, loaded in 9 chunks
    interleaved with src so compute starts after ~1.5MB.
  - Pixel -> partition map pi(r,c) = (r%4) + 64*(r//4) + 4c, so each
    pixel-row group's 16 partitions {p0(r)+4c} touch 8 SBUF AXI ports
    during the dump (a dense 16-partition group only reaches 4).
  - Device: per block, 2 fp16 matmuls (K=C=128, M=128 px, N=14 tgt rows
    x 26-36 ragged window cols) -> one 2-bank PSUM tile fp32; a single
    merged 4D DVE/ACT cast-copy per block -> fp16 staging, window-row-
    major [t:28][g:rows][wb][v] so each r-group's band (window rows
    r..r+20) is one contiguous 11-22KB run per partition.
  - Banded dump in supersteps of [1,1,2,2,1,1] hb rows (early start,
    small tail): one DMA per r-group spread over the sync HWDGE +
    gpsimd SWDGE (+ scalar on the tail) rings. 11.5MB/core total vs
    33MB dense fp32.
  - Host extracts (dy, dx) with per-wb strided views (the shear is
    unexpressible on-chip: BIR forbids mixed partition steps) and
    zeroes the structurally-out-of-image entries.
"""

import sys

if "/opt/trn_rl_repo" not in sys.path:
    sys.path.insert(0, "/opt/trn_rl_repo")

import numpy as np

B, NIN, C, H, W = 4, 2, 128, 128, 128
KH = KW = 21
QS = np.float32(1e10)
HHALF = 64            # rows per core
BH, BW = 8, 16        # pixel block (M = 128), partition = r*16 + c
NBH, NBW = HHALF // BH, W // BW      # 8, 8
RN = 14               # target rows per matmul (2 matmuls -> 28 = BH + 20)
WN = BW + 20          # 36 target cols per interior block
TR = BH + 20          # 28 window rows per block
# ragged window cols: edge blocks only cover the in-image part
WNS = [
    min(W, wb * BW + 26) - max(0, wb * BW - 10) for wb in range(NBW)
]                     # [26, 36*6, 26]
WC0 = [max(0, wb * BW - 10) for wb in range(NBW)]
WOFF = np.cumsum([0] + WNS).tolist()  # col offset inside a window row
ROWV = WOFF[-1]       # 268 elems per window row across a block row
SROW_F = TR * ROWV    # 7504 elems per partition per hb row
TROWS, TCOLS = HHALF + 20, W        # 84 rows (padded), 128 cols (no pad)
SRC_F = HHALF * W                    # 8192
TGT_F = TROWS * TCOLS                # 10752
PACK_F = SRC_F + TGT_F
NSS = NBH // 2                       # (legacy) 4 supersteps of 2 hb rows
SS_ROWS = [1, 2, 2, 1, 1, 1]         # hb rows per superstep
SS_HB0 = [0, 1, 3, 5, 6, 7]          # first hb of each superstep
# per-superstep band size per (r, c): 21 * g * ROWV elems
SS_BAND = [KH * g * ROWV for g in SS_ROWS]
SS_OFF = []
_o = 0
for _ss, _g in enumerate(SS_ROWS):
    SS_OFF.append(_o)
    _o += BH * BW * SS_BAND[_ss]
OUT_TOTAL = _o                       # 6193152 elems

# pixel (r, c) -> partition pi = (r%4) + 64*(r//4) + 4*c, so each r-group's
# 16 partitions {p0(r)+4c} hit 8 distinct SBUF AXI ports during the dump.
# _PIX_INV[m] = r*BW + c of the pixel stored in partition m.
_PIX_INV = np.array(
    [((m % 4) + 4 * (m // 64)) * BW + (m % 64) // 4 for m in range(128)],
    dtype=np.int64,
)

_nc_cache = None


def _build_nc():
    from contextlib import ExitStack

    from concourse import bacc, mybir, tile
    from concourse._compat import with_exitstack
    from concourse.ap import AP

    nc = bacc.Bacc("TRN2")
    pack = nc.declare_dram_parameter(
        "pack", [C, PACK_F], mybir.dt.float16, isOutput=False
    )
    # flat output: per superstep ss with g rows, 8 r-groups x 16 c x
    # (21 * g * 288) band elems
    out = nc.declare_dram_parameter(
        "out", [OUT_TOTAL], mybir.dt.float16, isOutput=True
    )

    # input chunks interleaved so each superstep's data lands just in time;
    # the first chunk is exactly what superstep 0 row 0 needs, to cut lead-in
    tgt_c = [
        (SRC_F + t0 * TCOLS, SRC_F + t1 * TCOLS)
        for t0, t1 in ((0, 28), (28, 44), (44, 60), (60, 76), (76, TROWS))
    ]
    assert TCOLS == W
    src_c = [(ss * 2048, (ss + 1) * 2048) for ss in range(NSS)]
    chunk_order = [
        tgt_c[0], src_c[0], tgt_c[1], src_c[1],
        tgt_c[2], src_c[2], tgt_c[3], src_c[3], tgt_c[4],
    ]

    @with_exitstack
    def kern(ctx: ExitStack, tc: tile.TileContext):
        nc = tc.nc
        sbp = ctx.enter_context(tc.tile_pool(name="inp", bufs=1))
        psp = ctx.enter_context(tc.tile_pool(name="ps", bufs=4, space="PSUM"))
        stg1 = ctx.enter_context(tc.tile_pool(name="stg1", bufs=2))
        stg2 = ctx.enter_context(tc.tile_pool(name="stg2", bufs=2))

        pk = sbp.tile([C, PACK_F], mybir.dt.float16, tag="pk")
        for lo, hi in chunk_order:
            nc.sync.dma_start(pk[:, lo:hi], pack[:, lo:hi])

        src2 = pk[:, 0:SRC_F]
        tgt3 = pk[:, SRC_F:].rearrange("c (t v) -> c t v", t=TROWS)

        copy_engines = [
            lambda d, s: nc.vector.tensor_copy(d, s),
            lambda d, s: nc.scalar.copy(d, s),
        ]
        ecnt = 0
        for ss, ng in enumerate(SS_ROWS):
            # staging layout per partition: [t:28][g:ng][wb:8][v:36]
            rowv2 = ng * ROWV
            srow = (stg1 if ng == 1 else stg2).tile(
                [128, ng * SROW_F], mybir.dt.float16
            )
            base = srow[:]
            pstride = base.ap[0][0]
            assert pstride == ng * SROW_F, (pstride, ng * SROW_F)
            for g in range(ng):
                hb = SS_HB0[ss] + g
                t0 = hb * BH
                for wb in range(NBW):
                    blk = hb * NBW + wb
                    wns, c0 = WNS[wb], WC0[wb]
                    nmm = RN * wns
                    lhs = src2[:, blk * 128 : (blk + 1) * 128]
                    ps = psp.tile([128, 1024], mybir.dt.float32)
                    nc.tensor.matmul(
                        ps[:, 0:nmm], lhs,
                        tgt3[:, t0 : t0 + RN, c0 : c0 + wns],
                        start=True, stop=True,
                    )
                    nc.tensor.matmul(
                        ps[:, 512 : 512 + nmm], lhs,
                        tgt3[:, t0 + RN : t0 + 2 * RN, c0 : c0 + wns],
                        start=True, stop=True,
                    )
                    # merged cast-copy: (p, half:2, t:14, v:wns)
                    src_ap = AP(
                        ps[:].tensor, ps[:].offset,
                        [[ps[:].ap[0][0], 128], [512, 2], [wns, RN], [1, wns]],
                    )
                    dst_ap = AP(
                        base.tensor,
                        base.offset + g * ROWV + WOFF[wb],
                        [[pstride, 128], [RN * rowv2, 2], [rowv2, RN], [1, wns]],
                    )
                    copy_engines[ecnt % 2](dst_ap, src_ap)
                    ecnt += 1
            # banded dump: r-group = 16 partitions {p0(r) + 4c}, ng rows
            # interleaved -> one contiguous (21*ng*288) run per partition.
            # The stride-4 partition scatter makes each DMA touch 8 SBUF
            # AXI ports (a dense 16-partition group only reaches 4).
            last = ss == len(SS_ROWS) - 1
            for r in range(BH):
                p0 = (r % 4) + 64 * (r // 4)
                band = AP(
                    base.tensor,
                    base.offset + p0 * pstride + r * rowv2,
                    [[4 * pstride, BW], [1, KH * rowv2]],
                )
                o0 = SS_OFF[ss] + r * BW * SS_BAND[ss]
                dst = out[o0 : o0 + BW * SS_BAND[ss]].rearrange(
                    "(c v) -> c v", c=BW
                )
                # spread across HWDGE(sync) + SWDGE(gpsimd) rings; on the
                # last superstep the scalar HWDGE ring joins (copies done)
                if last:
                    eng = (nc.sync, nc.gpsimd, nc.scalar)[r % 3]
                else:
                    eng = nc.sync if r % 2 == 0 else nc.gpsimd
                eng.dma_start(dst, band)

    with tile.TileContext(nc) as tc:
        kern(tc)
    nc.finalize()
    return nc


def _get_nc():
    global _nc_cache
    if _nc_cache is None:
        _nc_cache = _build_nc()
    return _nc_cache


def _pack_inputs(q: np.ndarray) -> list[dict]:
    """Per-core packed fp16 input: blocked src + zero-padded tgt."""
    in_maps = []
    for core in range(8):
        b, half = core // 2, core % 2
        h0 = half * HHALF
        src = q[b, 0, :, h0 : h0 + HHALF, :]            # (C, 64, 128)
        srcb = (
            src.reshape(C, NBH, BH, NBW, BW)
            .transpose(0, 1, 3, 2, 4)                   # (C, hb, wb, r, c)
            .reshape(C, NBH * NBW, BH * BW)[:, :, _PIX_INV]
            .reshape(C, SRC_F)
        )
        tgt = np.zeros((C, TROWS, TCOLS), np.float16)
        lo, hi = h0 - 10, h0 + HHALF + 10
        clo, chi = max(lo, 0), min(hi, H)
        tgt[:, clo - lo : chi - lo, :] = q[b, 1, :, clo:chi, :]
        pack = np.concatenate([srcb, tgt.reshape(C, TGT_F)], axis=1)
        in_maps.append({"pack": np.ascontiguousarray(pack)})
    return in_maps


def _unscramble(results: list[dict]) -> np.ndarray:
    """Extract the valid (dy, dx) band from each core's banded dump."""
    out = np.empty((B, KH * KW, H, W), np.float32)
    for core in range(8):
        b, half = core // 2, core % 2
        h0 = half * HHALF
        flat = np.asarray(results[core]["out"]).astype(np.float32)
        oc = np.zeros((KH, KW, HHALF, W), np.float32)
        for ss, ng in enumerate(SS_ROWS):
            # arr[r, c, u], u = dy*(ng*ROWV) + g*ROWV + WOFF[wb] + (c+dx-lo)
            arr = np.ascontiguousarray(
                flat[SS_OFF[ss] : SS_OFF[ss] + BH * BW * SS_BAND[ss]]
            ).reshape(BH, BW, SS_BAND[ss])
            # pad the band axis so edge-block shears stay in bounds
            arr = np.pad(arr, ((0, 0), (0, 0), (10, 10)))
            s_r, s_c, s_v = arr.strides
            h_lo = SS_HB0[ss] * BH
            for wb in range(NBW):
                lo_adj = wb * BW - 10 - WC0[wb]  # -10 for wb=0 else 0
                V = np.lib.stride_tricks.as_strided(
                    arr[:, :, 10 + WOFF[wb] + lo_adj :],
                    shape=(ng, BH, BW, KH, KW),
                    strides=(
                        ROWV * s_v, s_r, s_c + s_v,
                        ng * ROWV * s_v, s_v,
                    ),
                )
                # [g, r, c, dy, dx] -> [dy, dx, g, r, c]
                blkv = V.transpose(3, 4, 0, 1, 2).reshape(
                    KH, KW, ng * BH, BW
                )
                oc[:, :, h_lo : h_lo + ng * BH, wb * BW : (wb + 1) * BW] = blkv
        # zero structurally-invalid cols (displaced outside the image)
        wcol = np.arange(W)[None, :]
        dxs = np.arange(KW)[:, None]
        invalid = (wcol + dxs - 10 < 0) | (wcol + dxs - 10 >= W)  # [dx, w]
        oc *= ~invalid[None, :, None, :]
        out[b, :, h0 : h0 + HHALF, :] = oc.reshape(KH * KW, HHALF, W)
    out *= np.float32(1.0 / C)
    return out


def _run(inputs: np.ndarray, trace: bool = False, trace_kwargs: dict | None = None):
    from concourse.bass_utils import run_bass_kernel_spmd

    x = np.asarray(inputs, dtype=np.float32)
    assert x.shape == (B, NIN, C, H, W), x.shape
    q = (np.floor(x * QS) / QS).astype(np.float16)
    in_maps = _pack_inputs(q)
    nc = _get_nc()
    res = run_bass_kernel_spmd(
        nc, in_maps, core_ids=list(range(8)), trace=trace,
        **(trace_kwargs or {}),
    )
    out = _unscramble(res.results)
    return out, res


def kernel(inputs: np.ndarray) -> np.ndarray:
    out, _ = _run(inputs, trace=False)
    return out


# revision 49
# speedup vs baseline: 1.1125x; 1.0697x over previous
"""Correlation-volume kernel for Trainium2 (8 NeuronCores, SPMD).

Problem: inputs (B=4, N=2, C=128, H=128, W=128) fp32.
  q = floor(inputs * 1e10) / 1e10  (straight-through quantization, fp32)
  src = q[:, 0], tgt = q[:, 1]
  out[b, dy*21+dx, h, w] = mean_c src[b,c,h,w] * tgt[b,c,h+dy-10,w+dx-10]
  (zero padding outside), out shape (4, 441, 128, 128) fp32.

Strategy (final — fp16, ragged banded dump, port-scattered r-groups):
  - Shard batch(4) x H-half(2) across 8 cores, data parallel, no
    collectives. ~72us HW exec vs 131us for the fp32 dense-dump baseline.
  - Host precomputes q, casts fp16, blocks src into 128-pixel stationary
    tiles of 8 pixel-rows x 16 pixel-cols, zero-pads tgt rows only
    (84 x 128); one packed fp16 input per core, loaded in 9 chunks
    interleaved with src so compute starts after ~1.5MB.
  - Pixel -> partition map pi(r,c) = (r%4) + 64*(r//4) + 4c, so each
    pixel-row group's 16 partitions {p0(r)+4c} touch 8 SBUF AXI ports
    during the dump (a dense 16-partition group only reaches 4).
  - Device: per block, 2 fp16 matmuls (K=C=128, M=128 px, N=14 tgt rows
    x 26-36 ragged window cols) -> one 2-bank PSUM tile fp32; a single
    merged 4D DVE/ACT cast-copy per block -> fp16 staging, window-row-
    major [t:28][g:rows][wb][v] so each r-group's band (window rows
    r..r+20) is one contiguous 11-22KB run per partition.
  - Banded dump in supersteps of [1,1,2,2,1,1] hb rows (early start,
    small tail): one DMA per r-group spread over the sync HWDGE +
    gpsimd SWDGE (+ scalar on the tail) rings. 11.5MB/core total vs
    33MB dense fp32.
  - Host extracts (dy, dx) with per-wb strided views (the shear is
    unexpressible on-chip: BIR forbids mixed partition steps) and
    zeroes the structurally-out-of-image entries.
"""

import sys

if "/opt/trn_rl_repo" not in sys.path:
    sys.path.insert(0, "/opt/trn_rl_repo")

import numpy as np

B, NIN, C, H, W = 4, 2, 128, 128, 128
KH = KW = 21
QS = np.float32(1e10)
HHALF = 64            # rows per core
BH, BW = 8, 16        # pixel block (M = 128), partition = r*16 + c
NBH, NBW = HHALF // BH, W // BW      # 8, 8
RN = 14               # target rows per matmul (2 matmuls -> 28 = BH + 20)
WN = BW + 20          # 36 target cols per interior block
TR = BH + 20          # 28 window rows per block
# ragged window cols: edge blocks only cover the in-image part
WNS = [
    min(W, wb * BW + 26) - max(0, wb * BW - 10) for wb in range(NBW)
]                     # [26, 36*6, 26]
WC0 = [max(0, wb * BW - 10) for wb in range(NBW)]
WOFF = np.cumsum([0] + WNS).tolist()  # col offset inside a window row
ROWV = WOFF[-1]       # 268 elems per window row across a block row
SROW_F = TR * ROWV    # 7504 elems per partition per hb row
TROWS, TCOLS = HHALF + 20, W        # 84 rows (padded), 128 cols (no pad)
SRC_F = HHALF * W                    # 8192
TGT_F = TROWS * TCOLS                # 10752
PACK_F = SRC_F + TGT_F
NSS = NBH // 2                       # (legacy) 4 supersteps of 2 hb rows
SS_ROWS = [1, 1, 2, 2, 1, 1]         # hb rows per superstep
SS_HB0 = [0, 1, 2, 4, 6, 7]          # first hb of each superstep
# per-superstep band size per (r, c): 21 * g * ROWV elems
SS_BAND = [KH * g * ROWV for g in SS_ROWS]
SS_OFF = []
_o = 0
for _ss, _g in enumerate(SS_ROWS):
    SS_OFF.append(_o)
    _o += BH * BW * SS_BAND[_ss]
OUT_TOTAL = _o                       # 6193152 elems

# pixel (r, c) -> partition pi = (r%4) + 64*(r//4) + 4*c, so each r-group's
# 16 partitions {p0(r)+4c} hit 8 distinct SBUF AXI ports during the dump.
# _PIX_INV[m] = r*BW + c of the pixel stored in partition m.
_PIX_INV = np.array(
    [((m % 4) + 4 * (m // 64)) * BW + (m % 64) // 4 for m in range(128)],
    dtype=np.int64,
)

_nc_cache = None


def _build_nc():
    from contextlib import ExitStack

    from concourse import bacc, mybir, tile
    from concourse._compat import with_exitstack
    from concourse.ap import AP

    nc = bacc.Bacc("TRN2")
    pack = nc.declare_dram_parameter(
        "pack", [C, PACK_F], mybir.dt.float16, isOutput=False
    )
    # flat output: per superstep ss with g rows, 8 r-groups x 16 c x
    # (21 * g * 288) band elems
    out = nc.declare_dram_parameter(
        "out", [OUT_TOTAL], mybir.dt.float16, isOutput=True
    )

    # input chunks interleaved so each superstep's data lands just in time;
    # the first chunk is exactly what superstep 0 row 0 needs, to cut lead-in
    tgt_c = [
        (SRC_F + t0 * TCOLS, SRC_F + t1 * TCOLS)
        for t0, t1 in (
            (0, 14), (14, 28), (28, 44), (44, 60), (60, 76), (76, TROWS),
        )
    ]
    assert TCOLS == W
    src_c = [(ss * 2048, (ss + 1) * 2048) for ss in range(NSS)]
    chunk_order = [
        tgt_c[0], src_c[0], tgt_c[1], tgt_c[2], src_c[1],
        tgt_c[3], src_c[2], tgt_c[4], src_c[3], tgt_c[5],
    ]

    @with_exitstack
    def kern(ctx: ExitStack, tc: tile.TileContext):
        nc = tc.nc
        sbp = ctx.enter_context(tc.tile_pool(name="inp", bufs=1))
        psp = ctx.enter_context(tc.tile_pool(name="ps", bufs=4, space="PSUM"))
        stg1 = ctx.enter_context(tc.tile_pool(name="stg1", bufs=2))
        stg2 = ctx.enter_context(tc.tile_pool(name="stg2", bufs=2))

        pk = sbp.tile([C, PACK_F], mybir.dt.float16, tag="pk")
        for lo, hi in chunk_order:
            nc.sync.dma_start(pk[:, lo:hi], pack[:, lo:hi])

        src2 = pk[:, 0:SRC_F]
        tgt3 = pk[:, SRC_F:].rearrange("c (t v) -> c t v", t=TROWS)

        copy_engines = [
            lambda d, s: nc.vector.tensor_copy(d, s),
            lambda d, s: nc.scalar.copy(d, s),
        ]
        ecnt = 0
        for ss, ng in enumerate(SS_ROWS):
            # staging layout per partition: [t:28][g:ng][wb:8][v:36]
            rowv2 = ng * ROWV
            srow = (stg1 if ng == 1 else stg2).tile(
                [128, ng * SROW_F], mybir.dt.float16
            )
            base = srow[:]
            pstride = base.ap[0][0]
            assert pstride == ng * SROW_F, (pstride, ng * SROW_F)
            for g in range(ng):
                hb = SS_HB0[ss] + g
                t0 = hb * BH
                for wb in range(NBW):
                    blk = hb * NBW + wb
                    wns, c0 = WNS[wb], WC0[wb]
                    nmm = RN * wns
                    lhs = src2[:, blk * 128 : (blk + 1) * 128]
                    ps = psp.tile([128, 1024], mybir.dt.float32)
                    nc.tensor.matmul(
                        ps[:, 0:nmm], lhs,
                        tgt3[:, t0 : t0 + RN, c0 : c0 + wns],
                        start=True, stop=True,
                    )
                    nc.tensor.matmul(
                        ps[:, 512 : 512 + nmm], lhs,
                        tgt3[:, t0 + RN : t0 + 2 * RN, c0 : c0 + wns],
                        start=True, stop=True,
                    )
                    # merged cast-copy: (p, half:2, t:14, v:wns)
                    src_ap = AP(
                        ps[:].tensor, ps[:].offset,
                        [[ps[:].ap[0][0], 128], [512, 2], [wns, RN], [1, wns]],
                    )
                    dst_ap = AP(
                        base.tensor,
                        base.offset + g * ROWV + WOFF[wb],
                        [[pstride, 128], [RN * rowv2, 2], [rowv2, RN], [1, wns]],
                    )
                    copy_engines[ecnt % 2](dst_ap, src_ap)
                    ecnt += 1
            # banded dump: r-group = 16 partitions {p0(r) + 4c}, ng rows
            # interleaved -> one contiguous (21*ng*288) run per partition.
            # The stride-4 partition scatter makes each DMA touch 8 SBUF
            # AXI ports (a dense 16-partition group only reaches 4).
            last = ss >= len(SS_ROWS) - 2
            for r in range(BH):
                p0 = (r % 4) + 64 * (r // 4)
                band = AP(
                    base.tensor,
                    base.offset + p0 * pstride + r * rowv2,
                    [[4 * pstride, BW], [1, KH * rowv2]],
                )
                o0 = SS_OFF[ss] + r * BW * SS_BAND[ss]
                dst = out[o0 : o0 + BW * SS_BAND[ss]].rearrange(
                    "(c v) -> c v", c=BW
                )
                # spread across HWDGE(sync) + SWDGE(gpsimd) rings; on the
                # last two supersteps the scalar HWDGE ring joins
                # (its copies are nearly done by then)
                if last:
                    eng = (nc.sync, nc.gpsimd, nc.scalar)[r % 3]
                else:
                    eng = nc.sync if r % 2 == 0 else nc.gpsimd
                eng.dma_start(dst, band)

    with tile.TileContext(nc) as tc:
        kern(tc)
    nc.finalize()
    return nc


def _get_nc():
    global _nc_cache
    if _nc_cache is None:
        _nc_cache = _build_nc()
    return _nc_cache


def _pack_inputs(q: np.ndarray) -> list[dict]:
    """Per-core packed fp16 input: blocked src + zero-padded tgt."""
    in_maps = []
    for core in range(8):
        b, half = core // 2, core % 2
        h0 = half * HHALF
        src = q[b, 0, :, h0 : h0 + HHALF, :]            # (C, 64, 128)
        srcb = (
            src.reshape(C, NBH, BH, NBW, BW)
            .transpose(0, 1, 3, 2, 4)                   # (C, hb, wb, r, c)
            .reshape(C, NBH * NBW, BH * BW)[:, :, _PIX_INV]
            .reshape(C, SRC_F)
        )
        tgt = np.zeros((C, TROWS, TCOLS), np.float16)
        lo, hi = h0 - 10, h0 + HHALF + 10
        clo, chi = max(lo, 0), min(hi, H)
        tgt[:, clo - lo : chi - lo, :] = q[b, 1, :, clo:chi, :]
        pack = np.concatenate([srcb, tgt.reshape(C, TGT_F)], axis=1)
        in_maps.append({"pack": np.ascontiguousarray(pack)})
    return in_maps


def _unscramble(results: list[dict]) -> np.ndarray:
    """Extract the valid (dy, dx) band from each core's banded dump."""
    out = np.empty((B, KH * KW, H, W), np.float32)
    for core in range(8):
        b, half = core // 2, core % 2
        h0 = half * HHALF
        flat = np.asarray(results[core]["out"]).astype(np.float32)
        oc = np.zeros((KH, KW, HHALF, W), np.float32)
        for ss, ng in enumerate(SS_ROWS):
            # arr[r, c, u], u = dy*(ng*ROWV) + g*ROWV + WOFF[wb] + (c+dx-lo)
            arr = np.ascontiguousarray(
                flat[SS_OFF[ss] : SS_OFF[ss] + BH * BW * SS_BAND[ss]]
            ).reshape(BH, BW, SS_BAND[ss])
            # pad the band axis so edge-block shears stay in bounds
            arr = np.pad(arr, ((0, 0), (0, 0), (10, 10)))
            s_r, s_c, s_v = arr.strides
            h_lo = SS_HB0[ss] * BH
            for wb in range(NBW):
                lo_adj = wb * BW - 10 - WC0[wb]  # -10 for wb=0 else 0
                V = np.lib.stride_tricks.as_strided(
                    arr[:, :, 10 + WOFF[wb] + lo_adj :],
                    shape=(ng, BH, BW, KH, KW),
                    strides=(
                        ROWV * s_v, s_r, s_c + s_v,
                        ng * ROWV * s_v, s_v,
                    ),
                )
                # [g, r, c, dy, dx] -> [dy, dx, g, r, c]
                blkv = V.transpose(3, 4, 0, 1, 2).reshape(
                    KH, KW, ng * BH, BW
                )
                oc[:, :, h_lo : h_lo + ng * BH, wb * BW : (wb + 1) * BW] = blkv
        # zero structurally-invalid cols (displaced outside the image)
        wcol = np.arange(W)[None, :]
        dxs = np.arange(KW)[:, None]
        invalid = (wcol + dxs - 10 < 0) | (wcol + dxs - 10 >= W)  # [dx, w]
        oc *= ~invalid[None, :, None, :]
        out[b, :, h0 : h0 + HHALF, :] = oc.reshape(KH * KW, HHALF, W)
    out *= np.float32(1.0 / C)
    return out


def _run(inputs: np.ndarray, trace: bool = False, trace_kwargs: dict | None = None):
    from concourse.bass_utils import run_bass_kernel_spmd

    x = np.asarray(inputs, dtype=np.float32)
    assert x.shape == (B, NIN, C, H, W), x.shape
    q = (np.floor(x * QS) / QS).astype(np.float16)
    in_maps = _pack_inputs(q)
    nc = _get_nc()
    res = run_bass_kernel_spmd(
        nc, in_maps, core_ids=list(range(8)), trace=trace,
        **(trace_kwargs or {}),
    )
    out = _unscramble(res.results)
    return out, res


def kernel(inputs: np.ndarray) -> np.ndarray:
    out, _ = _run(inputs, trace=False)
    return out
